# revision 1
# baseline (speedup 1.0000x reference)
"""Multi-head attention (B=4, S=2048, D=1024, H=16) on 8 TRN2 NeuronCores.

Sharding: DP=4 over batch x TP=2 over heads. Core c handles batch c//2 and
heads 8*(c%2) .. 8*(c%2)+8. Each core computes a partial output [S, D] (its
heads' contribution to the out-projection); the host sums the two TP partials
per batch and adds the output bias.

Key compaction: the key-padding mask removes ~half the keys, so the host
gathers unmasked key rows per batch (padded to a multiple of 128). k/v
projections and attention only touch NKV ~= S/2 keys; padding keys carry a
-1e9 additive bias fused into the exp so they contribute exactly 0.

On-chip layouts (all matmul operands bf16, accumulation fp32 in PSUM):
  qT/kT : [hd, seq] with the two heads of a pair stacked on partitions
          (0-63 / 64-127) -> the scores matmuls (K=64) auto-pack into PE
          row-groups and run concurrently.
  scoresT[keys, q]: exp runs on ScalarE with fused scale + per-key mask bias,
          one op per [128, 1024] 2-bank PSUM tile covering both heads.
  v_aug : [keys, v | ones(64)] -> the ctx matmul accumulates ctxT (rows 0-63)
          and the softmax denominator replicated across rows 64-127, so the
          normalization is a shift-free fast-reciprocal + multiply on VectorE.

All inputs are host-packed into their SBUF layouts (partition-major, fully
contiguous per partition) so each tensor is one efficient DMA.
"""

import sys

sys.path.insert(0, "/opt/trn_rl_repo")

import numpy as np
import ml_dtypes

B, S, D, H = 4, 2048, 1024, 16
HD = D // H
SCALE = 1.0 / float(np.sqrt(HD))
NEG = -1e9

DP = 4  # batch shards
TP = 2  # head-group shards
HL = H // TP  # heads per core (8)
DL = HL * HD  # local head dims per core (512)
N_HP = HL // 2  # head pairs per core (4)
QCH = 512  # q chunk (free dim of score matmuls)
KV_P = 128  # key chunk (partition dim of scoresT)
KC8 = D // 128  # contraction chunks for projections (8)

bf16 = ml_dtypes.bfloat16


def _build(nkv, with_bias=True):
    from concourse import bacc
    import concourse.mybir as mybir
    from concourse.tile import TileContext

    dt = mybir.dt
    f32 = dt.float32
    b16 = dt.bfloat16
    EXP = mybir.ActivationFunctionType.Exp

    nkc = nkv // KV_P  # key chunks

    nc = bacc.Bacc(trn_type="TRN2")

    xq_d = nc.dram_tensor("xq", (128, KC8 * S), b16, kind="ExternalInput").ap()
    xkv_d = nc.dram_tensor("xkv", (128, KC8 * nkv), b16, kind="ExternalInput").ap()
    wq_d = nc.dram_tensor("wqt", (128, KC8 * DL), b16, kind="ExternalInput").ap()
    wk_d = nc.dram_tensor("wkt", (128, KC8 * DL), b16, kind="ExternalInput").ap()
    wv_d = nc.dram_tensor("wvt", (128, KC8 * DL), b16, kind="ExternalInput").ap()
    if with_bias:
        bq_d = nc.dram_tensor("bq", (1, DL), b16, kind="ExternalInput").ap()
        bk_d = nc.dram_tensor("bk", (1, DL), b16, kind="ExternalInput").ap()
        bv_d = nc.dram_tensor("bv", (1, DL), b16, kind="ExternalInput").ap()
    wo_d = nc.dram_tensor("wot", (128, (DL // 128) * D), b16, kind="ExternalInput").ap()
    mb_d = nc.dram_tensor("mbias", (nkv,), f32, kind="ExternalInput").ap()
    out_d = nc.dram_tensor("out", (S, D), f32, kind="ExternalOutput").ap()

    with TileContext(nc) as tc:
        with (
            tc.tile_pool(name="persist", bufs=1) as pp,
            tc.tile_pool(name="psum", bufs=2, space="PSUM") as ps2,
            tc.tile_pool(name="etile", bufs=8) as ep,
            tc.tile_pool(name="work", bufs=6) as wp,
            tc.tile_pool(name="ob", bufs=4) as obp,
        ):
            # ---- persistent SBUF tensors ----
            xq_sb = pp.tile([128, KC8, S], b16, tag="xq")
            xq1_sb = pp.tile([1, S], b16, tag="xq1")
            xkv_sb = pp.tile([128, KC8, nkv], b16, tag="xkv")
            xkv1_sb = pp.tile([1, nkv], b16, tag="xkv1")
            wq_sb = pp.tile([128, KC8, DL], b16, tag="wq")
            wk_sb = pp.tile([128, KC8, DL], b16, tag="wk")
            wv_sb = pp.tile([128, KC8, DL], b16, tag="wv")
            wq1_sb = pp.tile([1, DL], b16, tag="wq1")
            wk1_sb = pp.tile([1, DL], b16, tag="wk1")
            wv1_sb = pp.tile([1, DL], b16, tag="wv1")
            wo_sb = pp.tile([128, DL // 128, D], b16, tag="wo")
            mb_sb = pp.tile([128, nkc], f32, tag="mb")
            qt_sb = pp.tile([128, N_HP, S], b16, tag="qt")
            kt_sb = pp.tile([128, N_HP, nkv], b16, tag="kt")
            # v_aug: [key_part, key_chunk, head, 64 v | 64 ones]
            v_sb = pp.tile([128, nkc, HL, 128], b16, tag="v")
            ctx_sb = pp.tile([128, N_HP, S], b16, tag="ctx")

            # ---- loads (order gates the first matmuls: v-proj needs
            # wv+xkv, so those go first) ----
            # chunked loads: host-packed layout keeps every chunk fully
            # contiguous per partition, per-chunk DMAs keep deps fine-grained.
            # q/k weights + xq first: they gate the scores/exp pipeline start;
            # v-proj inputs follow and fill exp-paced bubbles.
            nc.sync.dma_start(mb_sb[:], mb_d.rearrange("(kc p) -> p kc", p=128))
            for kc in range(KC8):
                nc.sync.dma_start(
                    wq_sb[:, kc, :], wq_d[:, kc * DL : kc * DL + DL]
                )
                nc.sync.dma_start(
                    wk_sb[:, kc, :], wk_d[:, kc * DL : kc * DL + DL]
                )
                nc.sync.dma_start(xq_sb[:, kc, :], xq_d[:, kc * S : kc * S + S])
            if with_bias:
                nc.sync.dma_start(wq1_sb[:], bq_d)
                nc.sync.dma_start(wk1_sb[:], bk_d)
                nc.sync.dma_start(wv1_sb[:], bv_d)
            for kc in range(KC8):
                nc.sync.dma_start(
                    wv_sb[:, kc, :], wv_d[:, kc * DL : kc * DL + DL]
                )
                nc.sync.dma_start(
                    xkv_sb[:, kc, :], xkv_d[:, kc * nkv : kc * nkv + nkv]
                )
            for kc in range(DL // 128):
                nc.sync.dma_start(wo_sb[:, kc, :], wo_d[:, kc * D : kc * D + D])

            # constants: augmentation ones rows + the ones block of v_aug
            if with_bias:
                nc.vector.memset(xq1_sb[:], 1.0)
                nc.vector.memset(xkv1_sb[:], 1.0)
            nc.vector.memset(v_sb[:, :, :, 64:128], 1.0)

            def v_proj(mt):
                """v[keys mt*128:+128, all 8 heads] into v_sb."""
                ps = ps2.tile([128, DL], f32, tag="c")
                for kc in range(KC8):
                    nc.tensor.matmul(
                        ps[:],
                        lhsT=xkv_sb[:, kc, mt * 128 : mt * 128 + 128],
                        rhs=wv_sb[:, kc, :],
                        start=(kc == 0),
                        stop=(not with_bias and kc == KC8 - 1),
                    )
                if with_bias:
                    nc.tensor.matmul(
                        ps[:],
                        lhsT=xkv1_sb[:, mt * 128 : mt * 128 + 128],
                        rhs=wv1_sb[:],
                        start=False,
                        stop=True,
                    )
                nc.vector.tensor_copy(
                    out=v_sb[:, mt, :, 0:64],
                    in_=ps[:].rearrange("p (h e) -> p h e", h=HL),
                )

            def proj_qk(w_sb, w1_sb, dst_sb, hp, nfree):
                """dst[:, hp, :] = (w_aug.T @ x_aug) for head pair hp."""
                x_sb, x1_sb = (xq_sb, xq1_sb) if nfree == S else (xkv_sb, xkv1_sb)
                off = 0
                while off < nfree:
                    n = min(512, nfree - off)
                    ps = ps2.tile([128, 512], f32, tag="c")
                    for kc in range(KC8):
                        nc.tensor.matmul(
                            ps[:, :n],
                            lhsT=w_sb[:, kc, hp * 128 : hp * 128 + 128],
                            rhs=x_sb[:, kc, off : off + n],
                            start=(kc == 0),
                            stop=(not with_bias and kc == KC8 - 1),
                        )
                    if with_bias:
                        nc.tensor.matmul(
                            ps[:, :n],
                            lhsT=w1_sb[:, hp * 128 : hp * 128 + 128],
                            rhs=x1_sb[:, off : off + n],
                            start=False,
                            stop=True,
                        )
                    nc.vector.tensor_copy(
                        out=dst_sb[:, hp, off : off + n], in_=ps[:, :n]
                    )
                    off += n

            # head-pair 0 q/k first (gates the exp pipeline), then v
            proj_qk(wq_sb, wq1_sb, qt_sb, 0, S)
            proj_qk(wk_sb, wk1_sb, kt_sb, 0, nkv)
            for mt in range(nkc):
                v_proj(mt)

            # ---- q/k projections, interleaved with qc=0 attention so the
            # exp pipeline starts as soon as head-pair 0 is projected ----

            def outproj_mm(rt_abs):
                """Out-projection matmuls for row-tile rt_abs (128 rows)."""
                rs = slice(rt_abs * 128, rt_abs * 128 + 128)
                ps = ps2.tile([128, D], f32, tag="c")
                for nj in range(D // 512):
                    ns = slice(nj * 512, nj * 512 + 512)
                    for khp in range(N_HP):
                        nc.tensor.matmul(
                            ps[:, ns],
                            lhsT=ctx_sb[:, khp, rs],
                            rhs=wo_sb[:, khp, ns],
                            start=(khp == 0),
                            stop=(khp == N_HP - 1),
                        )
                return ps

            def outproj_out(rt_abs, ps):
                rs = slice(rt_abs * 128, rt_abs * 128 + 128)
                ob = obp.tile([128, D], f32, tag="ob")
                nc.vector.tensor_copy(out=ob[:], in_=ps[:])
                nc.sync.dma_start(out_d[rs, :], ob[:])

            # ---- attention (qc outer; the out-projection for q-chunk qc-1
            # is staggered into qc's hp iterations so its PSUM-slot use
            # never floods the attention pipeline) ----
            for qc in range(S // QCH):
                qs = slice(qc * QCH, qc * QCH + QCH)
                for hp in range(N_HP):
                    if qc == 0 and hp > 0:
                        proj_qk(wq_sb, wq1_sb, qt_sb, hp, S)
                        proj_qk(wk_sb, wk1_sb, kt_sb, hp, nkv)
                    # both heads' ctx accumulators in one 2-bank tile
                    cc = ps2.tile([128, 2 * QCH], f32, tag="c")
                    c0 = cc[:, 0:QCH]
                    c1 = cc[:, QCH : 2 * QCH]

                    def ctx_mm(ekc):
                        e01_p, kc_p = ekc
                        nc.tensor.matmul(
                            c0,
                            lhsT=v_sb[:, kc_p, 2 * hp, :],
                            rhs=e01_p[:, 0:QCH],
                            start=(kc_p == 0),
                            stop=(kc_p == nkc - 1),
                        )
                        nc.tensor.matmul(
                            c1,
                            lhsT=v_sb[:, kc_p, 2 * hp + 1, :],
                            rhs=e01_p[:, QCH : 2 * QCH],
                            start=(kc_p == 0),
                            stop=(kc_p == nkc - 1),
                        )

                    # software pipeline: ctx matmuls run two key-chunks behind
                    # the scores matmuls, so the exp latency is hidden and two
                    # ctx pairs drain at the hp boundary to cover the s-slot
                    # wait on the last exps
                    pending = []
                    for kc in range(nkc):
                        ks = slice(kc * KV_P, kc * KV_P + KV_P)
                        s01 = ps2.tile([128, 2 * QCH], f32, tag="s")
                        nc.tensor.matmul(
                            s01[:, 0:QCH],
                            lhsT=kt_sb[0:64, hp, ks],
                            rhs=qt_sb[0:64, hp, qs],
                        )
                        nc.tensor.matmul(
                            s01[:, QCH : 2 * QCH],
                            lhsT=kt_sb[64:128, hp, ks],
                            rhs=qt_sb[64:128, hp, qs],
                        )
                        e01 = ep.tile([128, 2 * QCH], b16, tag="e")
                        nc.scalar.activation(
                            e01[:],
                            s01[:],
                            EXP,
                            bias=mb_sb[:, kc : kc + 1],
                            scale=SCALE,
                        )
                        pending.append((e01, kc))
                        if len(pending) > 3:
                            ctx_mm(pending.pop(0))
                    for p in pending:
                        ctx_mm(p)

                    # previous q-chunk's out-projection matmuls right after
                    # the kc loop: they depend only on qc-1's ctx, so they
                    # fill the PE stall while ACT drains the last two exps.
                    # Their copy-out is emitted after the norm so the DVE
                    # queue frees this hp's ctx accumulator first.
                    op_ps = None
                    if qc > 0:
                        op_rt = (qc - 1) * (QCH // 128) + hp
                        op_ps = outproj_mm(op_rt)

                    # normalize: rows 64-127 hold the denominator (replicated).
                    # reciprocal_approx_fast (custom DVE) breaks on
                    # partition-shifted APs on HW, so first relocate the
                    # denominator to base partition 0 with a plain copy.
                    den0 = wp.tile([64, QCH], f32, tag="den")
                    nc.vector.tensor_copy(out=den0[:], in_=c0[64:128, :])
                    rc0 = wp.tile([64, QCH], f32, tag="rc")
                    nc.vector.reciprocal_approx_fast(rc0[:], den0[:])
                    nc.vector.tensor_mul(
                        out=ctx_sb[0:64, hp, qs], in0=c0[0:64, :], in1=rc0[:]
                    )
                    den1 = wp.tile([64, QCH], f32, tag="den")
                    nc.vector.tensor_copy(out=den1[:], in_=c1[64:128, :])
                    rc1 = wp.tile([64, QCH], f32, tag="rc")
                    nc.vector.reciprocal_approx_fast(rc1[:], den1[:])
                    nc.vector.tensor_mul(
                        out=ctx_sb[64:128, hp, qs], in0=c1[0:64, :], in1=rc1[:]
                    )

                    if op_ps is not None:
                        outproj_out(op_rt, op_ps)

            # out-projection for the last q-chunk's rows
            for rt in range(QCH // 128):
                rt_abs = (S // QCH - 1) * (QCH // 128) + rt
                outproj_out(rt_abs, outproj_mm(rt_abs))

    nc.finalize()
    return nc


def _pack(a, kc):
    """[kc*128, n] -> [128, kc*n] partition-major bf16 (SBUF layout)."""
    k128, n = a.shape
    return (
        np.ascontiguousarray(a.reshape(kc, 128, n).transpose(1, 0, 2))
        .reshape(128, kc * n)
        .astype(bf16)
    )


def _host_prep(x, mask, wq, bq, wk, bk, wv, bv, wo):
    x = np.asarray(x, dtype=np.float32)
    mask = np.asarray(mask)
    # per-batch gather of unmasked keys
    idxs = [np.nonzero(mask[b])[0] for b in range(B)]
    nmax = max(1, max(len(i) for i in idxs))
    nkv = min(S, ((nmax + KV_P - 1) // KV_P) * KV_P)
    with_bias = bool(
        np.any(np.asarray(bq)) or np.any(np.asarray(bk)) or np.any(np.asarray(bv))
    )

    in_maps = []
    for c in range(DP * TP):
        b, g = c // TP, c % TP
        sl = slice(g * DL, g * DL + DL)

        idx = idxs[b]
        xg = np.zeros((nkv, D), dtype=np.float32)
        xg[: len(idx)] = x[b][idx]

        mbias = np.full((nkv,), NEG, dtype=np.float32)
        mbias[: len(idx)] = 0.0

        im = {
            "xq": _pack(x[b].T, KC8),
            "xkv": _pack(xg.T, KC8),
            "wqt": _pack(np.asarray(wq)[sl, :].T, KC8),
            "wkt": _pack(np.asarray(wk)[sl, :].T, KC8),
            "wvt": _pack(np.asarray(wv)[sl, :].T, KC8),
            "wot": _pack(np.asarray(wo)[:, sl].T, DL // 128),
            "mbias": mbias,
        }
        if with_bias:
            im["bq"] = np.asarray(bq)[None, sl].astype(bf16)
            im["bk"] = np.asarray(bk)[None, sl].astype(bf16)
            im["bv"] = np.asarray(bv)[None, sl].astype(bf16)
        in_maps.append(im)
    return nkv, with_bias, in_maps


def kernel(x, mask, wq, bq, wk, bk, wv, bv, wo, bo):
    from concourse.bass_utils import run_bass_kernel_spmd

    nkv, with_bias, in_maps = _host_prep(x, mask, wq, bq, wk, bk, wv, bv, wo)
    nc = _build(nkv, with_bias)
    res = run_bass_kernel_spmd(nc, in_maps, core_ids=list(range(DP * TP)))

    out = np.empty((B, S, D), dtype=np.float32)
    bo = np.asarray(bo, dtype=np.float32)
    for b in range(B):
        out[b] = res.results[b * TP]["out"] + res.results[b * TP + 1]["out"] + bo
    return out



# revision 2
# speedup vs baseline: 1.0331x; 1.0331x over previous
"""Multi-head attention (B=4, S=2048, D=1024, H=16) on 8 TRN2 NeuronCores.

Sharding: DP=4 over batch x TP=2 over heads. Core c handles batch c//2 and
heads 8*(c%2) .. 8*(c%2)+8. Each core computes a partial output [S, D] (its
heads' contribution to the out-projection, bf16); the host sums the two TP
partials per batch in fp32 and adds the output bias.

Key compaction: the key-padding mask removes ~half the keys, so the host
gathers unmasked key rows per batch (padded to a multiple of 128). k/v
projections and attention only touch NKV ~= S/2 keys; padding keys carry a
-1e9 additive bias fused into the exp so they contribute exactly 0.

Schedule: the attention kc-loop (scores matmul pair -> exp on ACT -> ctx
matmul pair, software-pipelined) is the pacing spine. All projection and
out-projection matmuls are emitted as "filler" granules interleaved into the
spine so the PE works through them while ACT (the per-iteration pacer at
~1.15us/exp-tile vs ~0.85us of spine matmuls) keeps up. DMAs are issued from
both HWDGE queues (Sync + Scalar) in dependency order so the first exp fires
~12us in instead of ~43us.

On-chip layouts (all matmul operands bf16, accumulation fp32 in PSUM):
  qT/kT : [hd, seq] with the two heads of a pair stacked on partitions
          (0-63 / 64-127) -> the scores matmuls (K=64) pack into PE
          row-groups and run concurrently.
  scoresT[keys, q]: exp runs on ScalarE with fused scale + per-key mask bias,
          one op per [128, 1024] 2-bank PSUM tile covering both heads.
  v_aug : [keys, v | ones(64)] -> the ctx matmul accumulates ctxT (rows 0-63)
          and the softmax denominator replicated across rows 64-127, so the
          normalization is a shift-free fast-reciprocal + multiply on VectorE.
"""

import sys

sys.path.insert(0, "/opt/trn_rl_repo")

import numpy as np
import ml_dtypes

B, S, D, H = 4, 2048, 1024, 16
HD = D // H
SCALE = 1.0 / float(np.sqrt(HD))
NEG = -1e9

DP = 4  # batch shards
TP = 2  # head-group shards
HL = H // TP  # heads per core (8)
DL = HL * HD  # local head dims per core (512)
N_HP = HL // 2  # head pairs per core (4)
QCH = 512  # q chunk (free dim of score matmuls)
NQC = S // QCH  # 4
KC8 = D // 128  # contraction chunks for projections (8)
PIPE = 4  # ctx matmul pipeline depth (in kc iterations)

bf16 = ml_dtypes.bfloat16


def _build(nkv, with_bias=True):
    from concourse import bacc
    import concourse.mybir as mybir
    from concourse.tile import TileContext

    dt = mybir.dt
    f32 = dt.float32
    b16 = dt.bfloat16
    EXP = mybir.ActivationFunctionType.Exp

    nkc = nkv // 128  # key chunks (ctx contraction / scores output tiles)
    # k-projection free-dim chunks over the keys
    KOFF = []
    off = 0
    while off < nkv:
        n = min(512, nkv - off)
        KOFF.append((off, n))
        off += n

    nc = bacc.Bacc(trn_type="TRN2")

    xq_d = nc.dram_tensor("xq", (128, NQC * KC8 * QCH), b16, kind="ExternalInput").ap()
    xkv_d = nc.dram_tensor("xkv", (128, KC8 * nkv), b16, kind="ExternalInput").ap()
    wq_d = nc.dram_tensor("wqt", (128, N_HP * KC8 * 128), b16, kind="ExternalInput").ap()
    wk_d = nc.dram_tensor("wkt", (128, N_HP * KC8 * 128), b16, kind="ExternalInput").ap()
    wv_d = nc.dram_tensor("wvt", (128, KC8 * DL), b16, kind="ExternalInput").ap()
    wo_d = nc.dram_tensor("wot", (128, (DL // 128) * D), b16, kind="ExternalInput").ap()
    mb_d = nc.dram_tensor("mbias", (128, nkc), f32, kind="ExternalInput").ap()
    if with_bias:
        bq_d = nc.dram_tensor("bq", (1, DL), b16, kind="ExternalInput").ap()
        bk_d = nc.dram_tensor("bk", (1, DL), b16, kind="ExternalInput").ap()
        bv_d = nc.dram_tensor("bv", (1, DL), b16, kind="ExternalInput").ap()
    out_d = nc.dram_tensor("out", (S, D), b16, kind="ExternalOutput").ap()

    with TileContext(nc) as tc:
        with (
            tc.tile_pool(name="persist", bufs=1) as pp,
            tc.tile_pool(name="psA", bufs=2, space="PSUM") as spool,
            tc.tile_pool(name="psB", bufs=2, space="PSUM") as cpool,
            tc.tile_pool(name="etile", bufs=6) as ep,
            tc.tile_pool(name="work", bufs=6) as wp,
            tc.tile_pool(name="ob", bufs=3) as obp,
        ):
            # ---- persistent SBUF tensors ----
            xq_sb = pp.tile([128, NQC, KC8, QCH], b16, tag="xq")
            xkv_sb = pp.tile([128, KC8, nkv], b16, tag="xkv")
            wq_sb = pp.tile([128, N_HP, KC8, 128], b16, tag="wq")
            wk_sb = pp.tile([128, N_HP, KC8, 128], b16, tag="wk")
            wv_sb = pp.tile([128, KC8, DL], b16, tag="wv")
            wo_sb = pp.tile([128, DL // 128, D], b16, tag="wo")
            mb_sb = pp.tile([128, nkc], f32, tag="mb")
            qt_sb = pp.tile([128, N_HP, S], b16, tag="qt")
            kt_sb = pp.tile([128, N_HP, nkv], b16, tag="kt")
            # v_aug: [key_part, key_chunk, head, 64 v | 64 ones]
            v_sb = pp.tile([128, nkc, HL, 128], b16, tag="v")
            ctx_sb = pp.tile([128, N_HP, S], b16, tag="ctx")
            if with_bias:
                xq1_sb = pp.tile([1, S], b16, tag="xq1")
                xkv1_sb = pp.tile([1, nkv], b16, tag="xkv1")
                wq1_sb = pp.tile([1, DL], b16, tag="wq1")
                wk1_sb = pp.tile([1, DL], b16, tag="wk1")
                wv1_sb = pp.tile([1, DL], b16, tag="wv1")

            # ---- DMA issue plan: two HWDGE queues in dependency order ----
            # first scores need wk[hp0] + all xkv (k-proj contraction) +
            # wq[hp0] + xq[qc0]; v/wo trail.
            HPW = KC8 * 128  # per-hp weight cols (1024)
            nc.sync.dma_start(wk_sb[:, 0], wk_d[:, 0:HPW])
            nc.sync.dma_start(wq_sb[:, 0], wq_d[:, 0:HPW])
            nc.sync.dma_start(xq_sb[:, 0], xq_d[:, 0 : KC8 * QCH])
            for kc in (0, 2, 4, 6):
                nc.sync.dma_start(xkv_sb[:, kc], xkv_d[:, kc * nkv : (kc + 1) * nkv])
            for kc in (1, 3, 5, 7):
                nc.scalar.dma_start(xkv_sb[:, kc], xkv_d[:, kc * nkv : (kc + 1) * nkv])
            nc.scalar.dma_start(mb_sb[:], mb_d)
            if with_bias:
                nc.scalar.dma_start(wq1_sb[:], bq_d)
                nc.scalar.dma_start(wk1_sb[:], bk_d)
                nc.scalar.dma_start(wv1_sb[:], bv_d)
            # v-proj inputs next (needed ~4 iterations into hp0)
            half = KC8 // 2 * DL
            nc.sync.dma_start(wv_sb[:, 0 : KC8 // 2], wv_d[:, 0:half])
            nc.scalar.dma_start(wv_sb[:, KC8 // 2 :], wv_d[:, half:])
            # remaining q inputs + weights + wo
            for qc in (1, 2, 3):
                nc.sync.dma_start(
                    xq_sb[:, qc], xq_d[:, qc * KC8 * QCH : (qc + 1) * KC8 * QCH]
                )
            for hp in (1, 2, 3):
                nc.sync.dma_start(wk_sb[:, hp], wk_d[:, hp * HPW : (hp + 1) * HPW])
                nc.sync.dma_start(wq_sb[:, hp], wq_d[:, hp * HPW : (hp + 1) * HPW])
            nc.scalar.dma_start(wo_sb[:, 0:2], wo_d[:, 0 : 2 * D])
            nc.scalar.dma_start(wo_sb[:, 2:4], wo_d[:, 2 * D : 4 * D])

            # constants
            nc.vector.memset(v_sb[:, :, :, 64:128], 1.0)
            if with_bias:
                nc.vector.memset(xq1_sb[:], 1.0)
                nc.vector.memset(xkv1_sb[:], 1.0)

            # ---- granules ----
            def qproj(hp, qc):
                """qt[:, hp, qc*QCH:+QCH] = wq[hp].T @ xq[qc]."""
                qs = slice(qc * QCH, qc * QCH + QCH)
                ps = cpool.tile([128, 1024], f32, tag="c")
                for kc in range(KC8):
                    nc.tensor.matmul(
                        ps[:, 0:QCH],
                        lhsT=wq_sb[:, hp, kc, :],
                        rhs=xq_sb[:, qc, kc, :],
                        start=(kc == 0),
                        stop=(not with_bias and kc == KC8 - 1),
                    )
                if with_bias:
                    nc.tensor.matmul(
                        ps[:, 0:QCH],
                        lhsT=wq1_sb[:, hp * 128 : hp * 128 + 128],
                        rhs=xq1_sb[:, qs],
                        start=False,
                        stop=True,
                    )
                nc.vector.tensor_copy(out=qt_sb[:, hp, qs], in_=ps[:, 0:QCH])

            def kproj(hp, ko):
                """kt[:, hp, off:off+n] for key chunk ko."""
                off, n = KOFF[ko]
                ps = cpool.tile([128, 1024], f32, tag="c")
                for kc in range(KC8):
                    nc.tensor.matmul(
                        ps[:, 0:n],
                        lhsT=wk_sb[:, hp, kc, :],
                        rhs=xkv_sb[:, kc, off : off + n],
                        start=(kc == 0),
                        stop=(not with_bias and kc == KC8 - 1),
                    )
                if with_bias:
                    nc.tensor.matmul(
                        ps[:, 0:n],
                        lhsT=wk1_sb[:, hp * 128 : hp * 128 + 128],
                        rhs=xkv1_sb[:, off : off + n],
                        start=False,
                        stop=True,
                    )
                nc.vector.tensor_copy(
                    out=kt_sb[:, hp, off : off + n], in_=ps[:, 0:n]
                )

            def vproj(mt):
                """v[keys mt*128:+128, all 8 heads] into v_sb."""
                ps = cpool.tile([128, 1024], f32, tag="c")
                for kc in range(KC8):
                    nc.tensor.matmul(
                        ps[:, 0:DL],
                        lhsT=xkv_sb[:, kc, mt * 128 : mt * 128 + 128],
                        rhs=wv_sb[:, kc, :],
                        start=(kc == 0),
                        stop=(not with_bias and kc == KC8 - 1),
                    )
                if with_bias:
                    nc.tensor.matmul(
                        ps[:, 0:DL],
                        lhsT=xkv1_sb[:, mt * 128 : mt * 128 + 128],
                        rhs=wv1_sb[:],
                        start=False,
                        stop=True,
                    )
                nc.vector.tensor_copy(
                    out=v_sb[:, mt, :, 0:64],
                    in_=ps[:, 0:DL].rearrange("p (h e) -> p h e", h=HL),
                )

            def outproj(rt):
                """out rows rt*128:+128 = sum_khp ctx[khp].T @ wo[khp]."""
                rs = slice(rt * 128, rt * 128 + 128)
                ps = cpool.tile([128, 1024], f32, tag="c")
                for nj in range(D // 512):
                    ns = slice(nj * 512, nj * 512 + 512)
                    for khp in range(N_HP):
                        nc.tensor.matmul(
                            ps[:, ns],
                            lhsT=ctx_sb[:, khp, rs],
                            rhs=wo_sb[:, khp, ns],
                            start=(khp == 0),
                            stop=(khp == N_HP - 1),
                        )
                ob = obp.tile([128, D], b16, tag="ob")
                nc.vector.tensor_copy(out=ob[:], in_=ps[:])
                nc.sync.dma_start(out_d[rs, :], ob[:])

            # granule bookkeeping: ensure() emits on demand; the filler deque
            # paces the rest into the attention spine.
            done = set()
            FN = {"q": qproj, "k": kproj, "v": vproj, "o": outproj}

            def ensure(kind, *a):
                key = (kind,) + a
                if key not in done:
                    done.add(key)
                    FN[kind](*a)

            fillers = []

            def drain(n):
                while n > 0 and fillers:
                    key = fillers.pop(0)
                    if key in done:
                        continue
                    done.add(key)
                    FN[key[0]](*key[1:])
                    n -= 1

            # lead-in: enough q/k for the first scores
            ensure("k", 0, 0)
            ensure("q", 0, 0)

            # filler order ~ by deadline. v chunks first (ctx consumes them
            # from ~iteration PIPE of hp0), k chunks for hp0 next, then later
            # head-pairs' q/k, then q for later q-chunks.
            for mt in range(2):
                fillers.append(("v", mt))
            fillers.append(("k", 0, 1))
            for mt in range(2, 5):
                fillers.append(("v", mt))
            if len(KOFF) > 2:
                fillers.append(("k", 0, 2))
            for mt in range(5, nkc):
                fillers.append(("v", mt))
            for hp in (1, 2, 3):
                fillers.append(("q", hp, 0))
                for ko in range(len(KOFF)):
                    fillers.append(("k", hp, ko))
            for qc in (1, 2, 3):
                for hp in range(N_HP):
                    fillers.append(("q", hp, qc))

            # ---- attention spine ----
            for qc in range(NQC):
                qs = slice(qc * QCH, qc * QCH + QCH)
                for hp in range(N_HP):
                    ensure("q", hp, qc)
                    for ko in range(len(KOFF)):
                        ensure("k", hp, ko)

                    cc = cpool.tile([128, 1024], f32, tag="c")
                    c0 = cc[:, 0:QCH]
                    c1 = cc[:, QCH : 2 * QCH]

                    def ctx_mm(ekc):
                        e01_p, kc_p = ekc
                        ensure("v", kc_p)
                        nc.tensor.matmul(
                            c0,
                            lhsT=v_sb[:, kc_p, 2 * hp, :],
                            rhs=e01_p[:, 0:QCH],
                            start=(kc_p == 0),
                            stop=(kc_p == nkc - 1),
                        )
                        nc.tensor.matmul(
                            c1,
                            lhsT=v_sb[:, kc_p, 2 * hp + 1, :],
                            rhs=e01_p[:, QCH : 2 * QCH],
                            start=(kc_p == 0),
                            stop=(kc_p == nkc - 1),
                        )

                    pending = []
                    for kc in range(nkc):
                        ks = slice(kc * 128, kc * 128 + 128)
                        s01 = spool.tile([128, 2 * QCH], f32, tag="s")
                        nc.tensor.matmul(
                            s01[:, 0:QCH],
                            lhsT=kt_sb[0:64, hp, ks],
                            rhs=qt_sb[0:64, hp, qs],
                        )
                        nc.tensor.matmul(
                            s01[:, QCH : 2 * QCH],
                            lhsT=kt_sb[64:128, hp, ks],
                            rhs=qt_sb[64:128, hp, qs],
                        )
                        e01 = ep.tile([128, 2 * QCH], b16, tag="e")
                        nc.scalar.activation(
                            e01[:],
                            s01[:],
                            EXP,
                            bias=mb_sb[:, kc : kc + 1],
                            scale=SCALE,
                        )
                        pending.append((e01, kc))
                        if len(pending) >= PIPE:
                            ctx_mm(pending.pop(0))
                        # pace projection/out-proj work into the ACT slack
                        if qc == 0 or (kc & 1) == 0:
                            drain(1)
                    for p in pending:
                        ctx_mm(p)

                    # normalize: rows 64-127 hold the denominator (replicated).
                    # reciprocal_approx_fast needs base partition 0, so first
                    # relocate the denominator with a plain copy.
                    den0 = wp.tile([64, QCH], f32, tag="den")
                    nc.vector.tensor_copy(out=den0[:], in_=c0[64:128, :])
                    rc0 = wp.tile([64, QCH], f32, tag="rc")
                    nc.vector.reciprocal_approx_fast(rc0[:], den0[:])
                    nc.vector.tensor_mul(
                        out=ctx_sb[0:64, hp, qs], in0=c0[0:64, :], in1=rc0[:]
                    )
                    den1 = wp.tile([64, QCH], f32, tag="den")
                    nc.vector.tensor_copy(out=den1[:], in_=c1[64:128, :])
                    rc1 = wp.tile([64, QCH], f32, tag="rc")
                    nc.vector.reciprocal_approx_fast(rc1[:], den1[:])
                    nc.vector.tensor_mul(
                        out=ctx_sb[64:128, hp, qs], in0=c1[0:64, :], in1=rc1[:]
                    )

                    # out-projection of the previous q-chunk's row tile hp
                    if qc > 0:
                        ensure("o", (qc - 1) * (QCH // 128) + hp)

            # out-projection for the last q-chunk's rows + any stragglers
            drain(len(fillers))
            for rt in range((NQC - 1) * (QCH // 128), NQC * (QCH // 128)):
                ensure("o", rt)

    nc.finalize()
    return nc


def _host_prep(x, mask, wq, bq, wk, bk, wv, bv, wo):
    x = np.asarray(x, dtype=np.float32)
    mask = np.asarray(mask)
    # per-batch gather of unmasked keys
    idxs = [np.nonzero(mask[b])[0] for b in range(B)]
    nmax = max(1, max(len(i) for i in idxs))
    nkv = min(S, ((nmax + 127) // 128) * 128)
    nkc = nkv // 128
    with_bias = bool(
        np.any(np.asarray(bq)) or np.any(np.asarray(bk)) or np.any(np.asarray(bv))
    )

    in_maps = []
    for c in range(DP * TP):
        b, g = c // TP, c % TP
        sl = slice(g * DL, g * DL + DL)

        idx = idxs[b]
        xg = np.zeros((nkv, D), dtype=np.float32)
        xg[: len(idx)] = x[b][idx]

        mbias = np.full((nkv,), NEG, dtype=np.float32)
        mbias[: len(idx)] = 0.0

        # xq: [128, qc, kc, 512] partition-major
        xqp = np.ascontiguousarray(
            x[b].reshape(NQC, QCH, KC8, 128).transpose(3, 0, 2, 1)
        ).reshape(128, -1).astype(bf16)
        # xkv: [128, kc, nkv]
        xkvp = np.ascontiguousarray(
            xg.reshape(nkv, KC8, 128).transpose(2, 1, 0)
        ).reshape(128, -1).astype(bf16)
        # wq/wk: [128, hp, kc, 128] (w.T chunked partition-major, hp-major)
        def packw(w):
            t = np.asarray(w)[sl, :].T  # [D, DL]
            return np.ascontiguousarray(
                t.reshape(KC8, 128, N_HP, 128).transpose(1, 2, 0, 3)
            ).reshape(128, -1).astype(bf16)
        # wv: [128, kc, DL]
        wvp = np.ascontiguousarray(
            np.asarray(wv)[sl, :].T.reshape(KC8, 128, DL).transpose(1, 0, 2)
        ).reshape(128, -1).astype(bf16)
        # wo: [128, DL//128, D]
        wop = np.ascontiguousarray(
            np.asarray(wo)[:, sl].T.reshape(DL // 128, 128, D).transpose(1, 0, 2)
        ).reshape(128, -1).astype(bf16)

        im = {
            "xq": xqp,
            "xkv": xkvp,
            "wqt": packw(wq),
            "wkt": packw(wk),
            "wvt": wvp,
            "wot": wop,
            "mbias": np.ascontiguousarray(mbias.reshape(nkc, 128).T),
        }
        if with_bias:
            im["bq"] = np.asarray(bq)[None, sl].astype(bf16)
            im["bk"] = np.asarray(bk)[None, sl].astype(bf16)
            im["bv"] = np.asarray(bv)[None, sl].astype(bf16)
        in_maps.append(im)
    return nkv, with_bias, in_maps


def kernel(x, mask, wq, bq, wk, bk, wv, bv, wo, bo):
    from concourse.bass_utils import run_bass_kernel_spmd

    nkv, with_bias, in_maps = _host_prep(x, mask, wq, bq, wk, bk, wv, bv, wo)
    nc = _build(nkv, with_bias)
    res = run_bass_kernel_spmd(nc, in_maps, core_ids=list(range(DP * TP)))

    out = np.empty((B, S, D), dtype=np.float32)
    bo = np.asarray(bo, dtype=np.float32)
    for b in range(B):
        out[b] = (
            res.results[b * TP]["out"].astype(np.float32)
            + res.results[b * TP + 1]["out"].astype(np.float32)
            + bo
        )
    return out


# revision 6
# speedup vs baseline: 1.0538x; 1.0200x over previous
"""Multi-head attention (B=4, S=2048, D=1024, H=16) on 8 TRN2 NeuronCores.

Sharding: DP=4 over batch x TP=2 over heads. Core c handles batch c//2 and
heads 8*(c%2) .. 8*(c%2)+8. Each core computes a partial output [S, D] (its
heads' contribution to the out-projection, bf16); the host sums the two TP
partials per batch in fp32 and adds the output bias.

Key compaction: the key-padding mask removes ~half the keys, so the host
gathers unmasked key rows per batch (padded to a multiple of 128). k/v
projections and attention only touch NKV ~= S/2 keys; padding keys carry a
-1e9 additive bias fused into the exp so they contribute exactly 0.

Schedule: the attention kc-loop (scores matmul pair -> exp on ACT -> ctx
matmul pair, software-pipelined) is the pacing spine. All projection and
out-projection matmuls are emitted as "filler" granules interleaved into the
spine so the PE works through them while ACT (the per-iteration pacer at
~1.15us/exp-tile vs ~0.85us of spine matmuls) keeps up. DMAs are issued from
both HWDGE queues (Sync + Scalar) in dependency order so the first exp fires
~12us in instead of ~43us.

On-chip layouts (all matmul operands bf16, accumulation fp32 in PSUM):
  qT/kT : [hd, seq] with the two heads of a pair stacked on partitions
          (0-63 / 64-127) -> the scores matmuls (K=64) pack into PE
          row-groups and run concurrently.
  scoresT[keys, q]: exp runs on ScalarE with fused scale + per-key mask bias,
          one op per [128, 1024] 2-bank PSUM tile covering both heads.
  v_aug : [keys, v | ones(64)] -> the ctx matmul accumulates ctxT (rows 0-63)
          and the softmax denominator replicated across rows 64-127, so the
          normalization is a shift-free fast-reciprocal + multiply on VectorE.
"""

import sys

sys.path.insert(0, "/opt/trn_rl_repo")

import numpy as np
import ml_dtypes

B, S, D, H = 4, 2048, 1024, 16
HD = D // H
SCALE = 1.0 / float(np.sqrt(HD))
NEG = -1e9

DP = 4  # batch shards
TP = 2  # head-group shards
HL = H // TP  # heads per core (8)
DL = HL * HD  # local head dims per core (512)
N_HP = HL // 2  # head pairs per core (4)
QCH = 512  # q chunk (free dim of score matmuls)
NQC = S // QCH  # 4
KC8 = D // 128  # contraction chunks for projections (8)
PIPE = 4  # ctx matmul pipeline depth (in kc iterations)

bf16 = ml_dtypes.bfloat16


def _build(nkv, with_bias=True):
    from concourse import bacc
    import concourse.mybir as mybir
    from concourse.tile import TileContext

    dt = mybir.dt
    f32 = dt.float32
    b16 = dt.bfloat16
    EXP = mybir.ActivationFunctionType.Exp

    nkc = nkv // 128  # key chunks (ctx contraction / scores output tiles)
    # k-projection free-dim chunks over the keys
    KOFF = []
    off = 0
    while off < nkv:
        n = min(512, nkv - off)
        KOFF.append((off, n))
        off += n

    nc = bacc.Bacc(trn_type="TRN2")

    xq_d = nc.dram_tensor("xq", (128, NQC * KC8 * QCH), b16, kind="ExternalInput").ap()
    xkv_d = nc.dram_tensor("xkv", (128, KC8 * nkv), b16, kind="ExternalInput").ap()
    wq_d = nc.dram_tensor("wqt", (128, N_HP * KC8 * 128), b16, kind="ExternalInput").ap()
    wk_d = nc.dram_tensor("wkt", (128, N_HP * KC8 * 128), b16, kind="ExternalInput").ap()
    wv_d = nc.dram_tensor("wvt", (128, KC8 * DL), b16, kind="ExternalInput").ap()
    wo_d = nc.dram_tensor("wot", (128, (DL // 128) * D), b16, kind="ExternalInput").ap()
    mb_d = nc.dram_tensor("mbias", (128, nkc), f32, kind="ExternalInput").ap()
    if with_bias:
        bq_d = nc.dram_tensor("bq", (1, DL), b16, kind="ExternalInput").ap()
        bk_d = nc.dram_tensor("bk", (1, DL), b16, kind="ExternalInput").ap()
        bv_d = nc.dram_tensor("bv", (1, DL), b16, kind="ExternalInput").ap()
    out_d = nc.dram_tensor("out", (S, D), b16, kind="ExternalOutput").ap()

    with TileContext(nc) as tc:
        with (
            tc.tile_pool(name="persist", bufs=1) as pp,
            tc.tile_pool(name="psA", bufs=2, space="PSUM") as spool,
            tc.tile_pool(name="psB", bufs=2, space="PSUM") as cpool,
            tc.tile_pool(name="etile", bufs=8) as ep,
            tc.tile_pool(name="work", bufs=6) as wp,
            tc.tile_pool(name="ob", bufs=3) as obp,
        ):
            # ---- persistent SBUF tensors ----
            xq_sb = pp.tile([128, NQC, KC8, QCH], b16, tag="xq")
            xkv_sb = pp.tile([128, KC8, nkv], b16, tag="xkv")
            wq_sb = pp.tile([128, N_HP, KC8, 128], b16, tag="wq")
            wk_sb = pp.tile([128, N_HP, KC8, 128], b16, tag="wk")
            wv_sb = pp.tile([128, KC8, DL], b16, tag="wv")
            wo_sb = pp.tile([128, DL // 128, D], b16, tag="wo")
            mb_sb = pp.tile([128, nkc], f32, tag="mb")
            qt_sb = pp.tile([128, N_HP, S], b16, tag="qt")
            kt_sb = pp.tile([128, N_HP, nkv], b16, tag="kt")
            # v_aug: [key_part, key_chunk, head, 64 v | 64 ones]
            v_sb = pp.tile([128, nkc, HL, 128], b16, tag="v")
            ctx_sb = pp.tile([128, N_HP, S], b16, tag="ctx")
            if with_bias:
                xq1_sb = pp.tile([1, S], b16, tag="xq1")
                xkv1_sb = pp.tile([1, nkv], b16, tag="xkv1")
                wq1_sb = pp.tile([1, DL], b16, tag="wq1")
                wk1_sb = pp.tile([1, DL], b16, tag="wk1")
                wv1_sb = pp.tile([1, DL], b16, tag="wv1")

            # ---- DMA issue plan: two HWDGE queues, strict need-order ----
            # aggregate HBM read bw is the startup constraint (~300 GB/s),
            # so bytes are ordered exactly by first-use time:
            # wk0+xkv (k-proj) -> wq0+xq0 (q-proj) -> wv (v-proj fillers)
            # -> wk/wq hp1..3 + xq qc1..3 -> wo (out-proj, first use ~70us).
            HPW = KC8 * 128  # per-hp weight cols (1024)
            nc.sync.dma_start(wk_sb[:, 0], wk_d[:, 0:HPW])
            for kc in (0, 2, 4, 6):
                nc.sync.dma_start(xkv_sb[:, kc], xkv_d[:, kc * nkv : (kc + 1) * nkv])
            nc.scalar.dma_start(mb_sb[:], mb_d)
            if with_bias:
                nc.scalar.dma_start(wq1_sb[:], bq_d)
                nc.scalar.dma_start(wk1_sb[:], bk_d)
                nc.scalar.dma_start(wv1_sb[:], bv_d)
            for kc in (1, 3, 5, 7):
                nc.scalar.dma_start(xkv_sb[:, kc], xkv_d[:, kc * nkv : (kc + 1) * nkv])
            nc.sync.dma_start(wq_sb[:, 0], wq_d[:, 0:HPW])
            nc.sync.dma_start(xq_sb[:, 0], xq_d[:, 0 : KC8 * QCH])
            half = KC8 // 2 * DL
            nc.scalar.dma_start(wv_sb[:, KC8 // 2 :], wv_d[:, half:])
            nc.sync.dma_start(wv_sb[:, 0 : KC8 // 2], wv_d[:, 0:half])
            nc.sync.dma_start(wk_sb[:, 1], wk_d[:, HPW : 2 * HPW])
            nc.sync.dma_start(wq_sb[:, 1], wq_d[:, HPW : 2 * HPW])
            nc.sync.dma_start(xq_sb[:, 1], xq_d[:, KC8 * QCH : 2 * KC8 * QCH])
            for hp in (2, 3):
                nc.sync.dma_start(wk_sb[:, hp], wk_d[:, hp * HPW : (hp + 1) * HPW])
                nc.sync.dma_start(wq_sb[:, hp], wq_d[:, hp * HPW : (hp + 1) * HPW])
            for qc in (2, 3):
                nc.sync.dma_start(
                    xq_sb[:, qc], xq_d[:, qc * KC8 * QCH : (qc + 1) * KC8 * QCH]
                )
            nc.scalar.dma_start(wo_sb[:, 0:2], wo_d[:, 0 : 2 * D])
            nc.scalar.dma_start(wo_sb[:, 2:4], wo_d[:, 2 * D : 4 * D])

            # constants
            nc.vector.memset(v_sb[:, :, :, 64:128], 1.0)
            if with_bias:
                nc.vector.memset(xq1_sb[:], 1.0)
                nc.vector.memset(xkv1_sb[:], 1.0)

            # ---- granules ----
            def qproj(hp, qc):
                """qt[:, hp, qc*QCH:+QCH] = wq[hp].T @ xq[qc]."""
                qs = slice(qc * QCH, qc * QCH + QCH)
                ps = cpool.tile([128, 1024], f32, tag="c")
                for kc in range(KC8):
                    nc.tensor.matmul(
                        ps[:, 0:QCH],
                        lhsT=wq_sb[:, hp, kc, :],
                        rhs=xq_sb[:, qc, kc, :],
                        start=(kc == 0),
                        stop=(not with_bias and kc == KC8 - 1),
                    )
                if with_bias:
                    nc.tensor.matmul(
                        ps[:, 0:QCH],
                        lhsT=wq1_sb[:, hp * 128 : hp * 128 + 128],
                        rhs=xq1_sb[:, qs],
                        start=False,
                        stop=True,
                    )
                nc.vector.tensor_copy(out=qt_sb[:, hp, qs], in_=ps[:, 0:QCH])

            def kproj(hp, ko):
                """kt[:, hp, off:off+n] for key chunk ko."""
                off, n = KOFF[ko]
                ps = cpool.tile([128, 1024], f32, tag="c")
                for kc in range(KC8):
                    nc.tensor.matmul(
                        ps[:, 0:n],
                        lhsT=wk_sb[:, hp, kc, :],
                        rhs=xkv_sb[:, kc, off : off + n],
                        start=(kc == 0),
                        stop=(not with_bias and kc == KC8 - 1),
                    )
                if with_bias:
                    nc.tensor.matmul(
                        ps[:, 0:n],
                        lhsT=wk1_sb[:, hp * 128 : hp * 128 + 128],
                        rhs=xkv1_sb[:, off : off + n],
                        start=False,
                        stop=True,
                    )
                nc.vector.tensor_copy(
                    out=kt_sb[:, hp, off : off + n], in_=ps[:, 0:n]
                )

            def vproj(mt):
                """v[keys mt*128:+128, all 8 heads] into v_sb."""
                ps = cpool.tile([128, 1024], f32, tag="c")
                for kc in range(KC8):
                    nc.tensor.matmul(
                        ps[:, 0:DL],
                        lhsT=xkv_sb[:, kc, mt * 128 : mt * 128 + 128],
                        rhs=wv_sb[:, kc, :],
                        start=(kc == 0),
                        stop=(not with_bias and kc == KC8 - 1),
                    )
                if with_bias:
                    nc.tensor.matmul(
                        ps[:, 0:DL],
                        lhsT=xkv1_sb[:, mt * 128 : mt * 128 + 128],
                        rhs=wv1_sb[:],
                        start=False,
                        stop=True,
                    )
                nc.vector.tensor_copy(
                    out=v_sb[:, mt, :, 0:64],
                    in_=ps[:, 0:DL].rearrange("p (h e) -> p h e", h=HL),
                )

            def outproj(rt):
                """out rows rt*128:+128 = sum_khp ctx[khp].T @ wo[khp]."""
                rs = slice(rt * 128, rt * 128 + 128)
                ps = cpool.tile([128, 1024], f32, tag="c")
                for nj in range(D // 512):
                    ns = slice(nj * 512, nj * 512 + 512)
                    for khp in range(N_HP):
                        nc.tensor.matmul(
                            ps[:, ns],
                            lhsT=ctx_sb[:, khp, rs],
                            rhs=wo_sb[:, khp, ns],
                            start=(khp == 0),
                            stop=(khp == N_HP - 1),
                        )
                ob = obp.tile([128, D], b16, tag="ob")
                nc.vector.tensor_copy(out=ob[:], in_=ps[:])
                nc.sync.dma_start(out_d[rs, :], ob[:])

            # granule bookkeeping: ensure() emits on demand; the filler deque
            # paces the rest into the attention spine.
            done = set()
            FN = {"q": qproj, "k": kproj, "v": vproj, "o": outproj}

            def ensure(kind, *a):
                key = (kind,) + a
                if key not in done:
                    done.add(key)
                    FN[kind](*a)

            fillers = []

            def drain(n):
                while n > 0 and fillers:
                    key = fillers.pop(0)
                    if key in done:
                        continue
                    done.add(key)
                    FN[key[0]](*key[1:])
                    n -= 1

            # lead-in: enough q/k for the first scores
            ensure("k", 0, 0)
            ensure("q", 0, 0)

            # filler order ~ by deadline. v chunks first (ctx consumes them
            # from ~iteration PIPE of hp0), k chunks for hp0 next, then later
            # head-pairs' q/k, then q for later q-chunks.
            fillers.extend(
                [("v", 0), ("v", 1), ("k", 0, 1), ("v", 2), ("v", 3)]
                + ([("k", 0, 2)] if len(KOFF) > 2 else [])
                + [("v", 4), ("v", 5), ("v", 6), ("q", 1, 0), ("v", 7)]
                + [("v", mt) for mt in range(8, nkc)]
            )
            for hp in (1, 2, 3):
                for ko in range(len(KOFF)):
                    fillers.append(("k", hp, ko))
                if hp < 3:
                    fillers.append(("q", hp + 1, 0))
            for qc in (1, 2, 3):
                for hp in range(N_HP):
                    fillers.append(("q", hp, qc))

            # ---- attention spine: one software pipeline across all blocks.
            # pending holds ctx-pair work and per-block norm markers; the
            # pipeline never flushes at block boundaries, so ACT keeps a
            # steady diet of exp tiles while the PE weaves ctx/norm/filler.
            pending = []  # entries: ("ctx", fn, e01, kc) | ("norm", fn)

            def pump():
                while sum(1 for p in pending if p[0] == "ctx") >= PIPE:
                    ent = pending.pop(0)
                    if ent[0] == "ctx":
                        ent[1](ent[2], ent[3])
                    else:
                        ent[1]()

            for qc in range(NQC):
                qs = slice(qc * QCH, qc * QCH + QCH)
                for hp in range(N_HP):
                    ensure("q", hp, qc)
                    blk = {}

                    def ctx_mm(e01_p, kc_p, blk=blk, hp=hp):
                        ensure("v", kc_p)
                        if "cc" not in blk:
                            blk["cc"] = cpool.tile(
                                [128, 1024], f32, tag="c", name="cc"
                            )
                        cc = blk["cc"]
                        nc.tensor.matmul(
                            cc[:, 0:QCH],
                            lhsT=v_sb[:, kc_p, 2 * hp, :],
                            rhs=e01_p[:, 0:QCH],
                            start=(kc_p == 0),
                            stop=(kc_p == nkc - 1),
                        )
                        nc.tensor.matmul(
                            cc[:, QCH : 2 * QCH],
                            lhsT=v_sb[:, kc_p, 2 * hp + 1, :],
                            rhs=e01_p[:, QCH : 2 * QCH],
                            start=(kc_p == 0),
                            stop=(kc_p == nkc - 1),
                        )

                    def norm(blk=blk, hp=hp, qs=qs, qc=qc):
                        # rows 64-127 of each half hold the denominator
                        # (replicated). reciprocal_approx_fast needs base
                        # partition 0, so relocate it with a plain copy first.
                        cc = blk["cc"]
                        for h in (0, 1):
                            ch = cc[:, h * QCH : (h + 1) * QCH]
                            den = wp.tile([64, QCH], f32, tag="den")
                            nc.vector.tensor_copy(out=den[:], in_=ch[64:128, :])
                            rc = wp.tile([64, QCH], f32, tag="rc")
                            nc.vector.reciprocal_approx_fast(rc[:], den[:])
                            nc.vector.tensor_mul(
                                out=ctx_sb[h * 64 : h * 64 + 64, hp, qs],
                                in0=ch[0:64, :],
                                in1=rc[:],
                            )
                        # out-projection of the previous q-chunk's row tile hp
                        if qc > 0:
                            ensure("o", (qc - 1) * (QCH // 128) + hp)

                    for kc in range(nkc):
                        ensure("k", hp, (kc * 128) // 512)
                        ks = slice(kc * 128, kc * 128 + 128)
                        s01 = spool.tile([128, 2 * QCH], f32, tag="s")
                        nc.tensor.matmul(
                            s01[:, 0:QCH],
                            lhsT=kt_sb[0:64, hp, ks],
                            rhs=qt_sb[0:64, hp, qs],
                        )
                        nc.tensor.matmul(
                            s01[:, QCH : 2 * QCH],
                            lhsT=kt_sb[64:128, hp, ks],
                            rhs=qt_sb[64:128, hp, qs],
                        )
                        e01 = ep.tile([128, 2 * QCH], b16, tag="e")
                        nc.scalar.activation(
                            e01[:],
                            s01[:],
                            EXP,
                            bias=mb_sb[:, kc : kc + 1],
                            scale=SCALE,
                        )
                        pending.append(("ctx", ctx_mm, e01, kc))
                        pump()
                        # pace projection/out-proj work into the ACT slack
                        if qc == 0:
                            drain(2)
                        elif (kc & 1) == 0:
                            drain(1)
                    pending.append(("norm", norm))

            # drain the pipeline, stragglers, and the last q-chunk's rows
            for ent in pending:
                if ent[0] == "ctx":
                    ent[1](ent[2], ent[3])
                else:
                    ent[1]()
            drain(len(fillers))
            for rt in range((NQC - 1) * (QCH // 128), NQC * (QCH // 128)):
                ensure("o", rt)

    nc.finalize()
    return nc


def _host_prep(x, mask, wq, bq, wk, bk, wv, bv, wo):
    x = np.asarray(x, dtype=np.float32)
    mask = np.asarray(mask)
    # per-batch gather of unmasked keys
    idxs = [np.nonzero(mask[b])[0] for b in range(B)]
    nmax = max(1, max(len(i) for i in idxs))
    nkv = min(S, ((nmax + 127) // 128) * 128)
    nkc = nkv // 128
    with_bias = bool(
        np.any(np.asarray(bq)) or np.any(np.asarray(bk)) or np.any(np.asarray(bv))
    )

    in_maps = []
    for c in range(DP * TP):
        b, g = c // TP, c % TP
        sl = slice(g * DL, g * DL + DL)

        idx = idxs[b]
        xg = np.zeros((nkv, D), dtype=np.float32)
        xg[: len(idx)] = x[b][idx]

        mbias = np.full((nkv,), NEG, dtype=np.float32)
        mbias[: len(idx)] = 0.0

        # xq: [128, qc, kc, 512] partition-major
        xqp = np.ascontiguousarray(
            x[b].reshape(NQC, QCH, KC8, 128).transpose(3, 0, 2, 1)
        ).reshape(128, -1).astype(bf16)
        # xkv: [128, kc, nkv]
        xkvp = np.ascontiguousarray(
            xg.reshape(nkv, KC8, 128).transpose(2, 1, 0)
        ).reshape(128, -1).astype(bf16)
        # wq/wk: [128, hp, kc, 128] (w.T chunked partition-major, hp-major)
        def packw(w):
            t = np.asarray(w)[sl, :].T  # [D, DL]
            return np.ascontiguousarray(
                t.reshape(KC8, 128, N_HP, 128).transpose(1, 2, 0, 3)
            ).reshape(128, -1).astype(bf16)
        # wv: [128, kc, DL]
        wvp = np.ascontiguousarray(
            np.asarray(wv)[sl, :].T.reshape(KC8, 128, DL).transpose(1, 0, 2)
        ).reshape(128, -1).astype(bf16)
        # wo: [128, DL//128, D]
        wop = np.ascontiguousarray(
            np.asarray(wo)[:, sl].T.reshape(DL // 128, 128, D).transpose(1, 0, 2)
        ).reshape(128, -1).astype(bf16)

        im = {
            "xq": xqp,
            "xkv": xkvp,
            "wqt": packw(wq),
            "wkt": packw(wk),
            "wvt": wvp,
            "wot": wop,
            "mbias": np.ascontiguousarray(mbias.reshape(nkc, 128).T),
        }
        if with_bias:
            im["bq"] = np.asarray(bq)[None, sl].astype(bf16)
            im["bk"] = np.asarray(bk)[None, sl].astype(bf16)
            im["bv"] = np.asarray(bv)[None, sl].astype(bf16)
        in_maps.append(im)
    return nkv, with_bias, in_maps


def kernel(x, mask, wq, bq, wk, bk, wv, bv, wo, bo):
    from concourse.bass_utils import run_bass_kernel_spmd

    nkv, with_bias, in_maps = _host_prep(x, mask, wq, bq, wk, bk, wv, bv, wo)
    nc = _build(nkv, with_bias)
    res = run_bass_kernel_spmd(nc, in_maps, core_ids=list(range(DP * TP)))

    out = np.empty((B, S, D), dtype=np.float32)
    bo = np.asarray(bo, dtype=np.float32)
    for b in range(B):
        out[b] = (
            res.results[b * TP]["out"].astype(np.float32)
            + res.results[b * TP + 1]["out"].astype(np.float32)
            + bo
        )
    return out


# revision 7
# speedup vs baseline: 1.0730x; 1.0183x over previous
"""Multi-head attention (B=4, S=2048, D=1024, H=16) on 8 TRN2 NeuronCores.

Sharding: DP=4 over batch x TP=2 over heads. Core c handles batch c//2 and
heads 8*(c%2) .. 8*(c%2)+8. Each core computes a partial output [S, D] (its
heads' contribution to the out-projection, bf16); the host sums the two TP
partials per batch in fp32 and adds the output bias.

Key compaction: the key-padding mask removes ~half the keys, so the host
gathers unmasked key rows per batch (padded to a multiple of 128). k/v
projections and attention only touch NKV ~= S/2 keys; padding keys carry a
-1e9 additive bias fused into the exp so they contribute exactly 0.

Schedule: the attention kc-loop (scores matmul pair -> exp on ACT -> ctx
matmul pair, software-pipelined) is the pacing spine. All projection and
out-projection matmuls are emitted as "filler" granules interleaved into the
spine so the PE works through them while ACT (the per-iteration pacer at
~1.15us/exp-tile vs ~0.85us of spine matmuls) keeps up. DMAs are issued from
both HWDGE queues (Sync + Scalar) in dependency order so the first exp fires
~12us in instead of ~43us.

On-chip layouts (all matmul operands bf16, accumulation fp32 in PSUM):
  qT/kT : [hd, seq] with the two heads of a pair stacked on partitions
          (0-63 / 64-127) -> the scores matmuls (K=64) pack into PE
          row-groups and run concurrently.
  scoresT[keys, q]: exp runs on ScalarE with fused scale + per-key mask bias,
          one op per [128, 1024] 2-bank PSUM tile covering both heads.
  v_aug : [keys, v | ones(64)] -> the ctx matmul accumulates ctxT (rows 0-63)
          and the softmax denominator replicated across rows 64-127, so the
          normalization is a shift-free fast-reciprocal + multiply on VectorE.
"""

import sys

sys.path.insert(0, "/opt/trn_rl_repo")

import numpy as np
import ml_dtypes

B, S, D, H = 4, 2048, 1024, 16
HD = D // H
SCALE = 1.0 / float(np.sqrt(HD))
NEG = -1e9

DP = 4  # batch shards
TP = 2  # head-group shards
HL = H // TP  # heads per core (8)
DL = HL * HD  # local head dims per core (512)
N_HP = HL // 2  # head pairs per core (4)
QCH = 512  # q chunk (free dim of score matmuls)
NQC = S // QCH  # 4
KC8 = D // 128  # contraction chunks for projections (8)
PIPE = 4  # ctx matmul pipeline depth (in kc iterations)

bf16 = ml_dtypes.bfloat16


def _build(nkv, with_bias=True):
    from concourse import bacc
    import concourse.mybir as mybir
    from concourse.tile import TileContext

    dt = mybir.dt
    f32 = dt.float32
    b16 = dt.bfloat16
    EXP = mybir.ActivationFunctionType.Exp

    nkc = nkv // 128  # key chunks (ctx contraction / scores output tiles)
    # k-projection free-dim chunks over the keys
    KOFF = []
    off = 0
    while off < nkv:
        n = min(512, nkv - off)
        KOFF.append((off, n))
        off += n

    nc = bacc.Bacc(trn_type="TRN2")

    xq_d = nc.dram_tensor("xq", (128, NQC * KC8 * QCH), b16, kind="ExternalInput").ap()
    xkv_d = nc.dram_tensor("xkv", (128, KC8 * nkv), b16, kind="ExternalInput").ap()
    wq_d = nc.dram_tensor("wqt", (128, N_HP * KC8 * 128), b16, kind="ExternalInput").ap()
    wk_d = nc.dram_tensor("wkt", (128, N_HP * KC8 * 128), b16, kind="ExternalInput").ap()
    wv_d = nc.dram_tensor("wvt", (128, KC8 * DL), b16, kind="ExternalInput").ap()
    wo_d = nc.dram_tensor("wot", (128, (DL // 128) * D), b16, kind="ExternalInput").ap()
    mb_d = nc.dram_tensor("mbias", (128, nkc), f32, kind="ExternalInput").ap()
    if with_bias:
        bq_d = nc.dram_tensor("bq", (1, DL), b16, kind="ExternalInput").ap()
        bk_d = nc.dram_tensor("bk", (1, DL), b16, kind="ExternalInput").ap()
        bv_d = nc.dram_tensor("bv", (1, DL), b16, kind="ExternalInput").ap()
    out_d = nc.dram_tensor("out", (S, D), b16, kind="ExternalOutput").ap()

    with TileContext(nc) as tc:
        with (
            tc.tile_pool(name="persist", bufs=1) as pp,
            tc.tile_pool(name="psA", bufs=2, space="PSUM") as spool,
            tc.tile_pool(name="psB", bufs=2, space="PSUM") as cpool,
            tc.tile_pool(name="etile", bufs=8) as ep,
            tc.tile_pool(name="work", bufs=6) as wp,
            tc.tile_pool(name="ob", bufs=3) as obp,
        ):
            # ---- persistent SBUF tensors ----
            xq_sb = pp.tile([128, NQC, KC8, QCH], b16, tag="xq")
            xkv_sb = pp.tile([128, KC8, nkv], b16, tag="xkv")
            wq_sb = pp.tile([128, N_HP, KC8, 128], b16, tag="wq")
            wk_sb = pp.tile([128, N_HP, KC8, 128], b16, tag="wk")
            wv_sb = pp.tile([128, KC8, DL], b16, tag="wv")
            wo_sb = pp.tile([128, DL // 128, D], b16, tag="wo")
            mb_sb = pp.tile([128, nkc], f32, tag="mb")
            qt_sb = pp.tile([128, N_HP, S], b16, tag="qt")
            kt_sb = pp.tile([128, N_HP, nkv], b16, tag="kt")
            # v_aug: [key_part, key_chunk, head, 64 v | 64 ones]
            v_sb = pp.tile([128, nkc, HL, 128], b16, tag="v")
            ctx_sb = pp.tile([128, N_HP, S], b16, tag="ctx")
            if with_bias:
                xq1_sb = pp.tile([1, S], b16, tag="xq1")
                xkv1_sb = pp.tile([1, nkv], b16, tag="xkv1")
                wq1_sb = pp.tile([1, DL], b16, tag="wq1")
                wk1_sb = pp.tile([1, DL], b16, tag="wk1")
                wv1_sb = pp.tile([1, DL], b16, tag="wv1")

            # ---- DMA issue plan: two HWDGE queues, strict need-order ----
            # aggregate HBM read bw is the startup constraint (~300 GB/s),
            # so bytes are ordered exactly by first-use time:
            # wk0+xkv (k-proj) -> wq0+xq0 (q-proj) -> wv (v-proj fillers)
            # -> wk/wq hp1..3 + xq qc1..3 -> wo (out-proj, first use ~70us).
            HPW = KC8 * 128  # per-hp weight cols (1024)
            nc.sync.dma_start(wk_sb[:, 0], wk_d[:, 0:HPW])
            for kc in (0, 2, 4, 6):
                nc.sync.dma_start(xkv_sb[:, kc], xkv_d[:, kc * nkv : (kc + 1) * nkv])
            nc.scalar.dma_start(mb_sb[:], mb_d)
            if with_bias:
                nc.scalar.dma_start(wq1_sb[:], bq_d)
                nc.scalar.dma_start(wk1_sb[:], bk_d)
                nc.scalar.dma_start(wv1_sb[:], bv_d)
            for kc in (1, 3, 5, 7):
                nc.scalar.dma_start(xkv_sb[:, kc], xkv_d[:, kc * nkv : (kc + 1) * nkv])
            nc.sync.dma_start(wq_sb[:, 0], wq_d[:, 0:HPW])
            nc.sync.dma_start(xq_sb[:, 0], xq_d[:, 0 : KC8 * QCH])
            half = KC8 // 2 * DL
            nc.scalar.dma_start(wv_sb[:, KC8 // 2 :], wv_d[:, half:])
            nc.sync.dma_start(wv_sb[:, 0 : KC8 // 2], wv_d[:, 0:half])
            nc.sync.dma_start(wk_sb[:, 1], wk_d[:, HPW : 2 * HPW])
            nc.sync.dma_start(wq_sb[:, 1], wq_d[:, HPW : 2 * HPW])
            nc.sync.dma_start(xq_sb[:, 1], xq_d[:, KC8 * QCH : 2 * KC8 * QCH])
            for hp in (2, 3):
                nc.sync.dma_start(wk_sb[:, hp], wk_d[:, hp * HPW : (hp + 1) * HPW])
                nc.sync.dma_start(wq_sb[:, hp], wq_d[:, hp * HPW : (hp + 1) * HPW])
            for qc in (2, 3):
                nc.sync.dma_start(
                    xq_sb[:, qc], xq_d[:, qc * KC8 * QCH : (qc + 1) * KC8 * QCH]
                )
            nc.scalar.dma_start(wo_sb[:, 0:2], wo_d[:, 0 : 2 * D])
            nc.scalar.dma_start(wo_sb[:, 2:4], wo_d[:, 2 * D : 4 * D])

            # constants
            nc.vector.memset(v_sb[:, :, :, 64:128], 1.0)
            if with_bias:
                nc.vector.memset(xq1_sb[:], 1.0)
                nc.vector.memset(xkv1_sb[:], 1.0)

            # ---- granules ----
            def qproj(hp, qc):
                """qt[:, hp, qc*QCH:+QCH] = wq[hp].T @ xq[qc]."""
                qs = slice(qc * QCH, qc * QCH + QCH)
                ps = cpool.tile([128, 1024], f32, tag="c")
                for kc in range(KC8):
                    nc.tensor.matmul(
                        ps[:, 0:QCH],
                        lhsT=wq_sb[:, hp, kc, :],
                        rhs=xq_sb[:, qc, kc, :],
                        start=(kc == 0),
                        stop=(not with_bias and kc == KC8 - 1),
                    )
                if with_bias:
                    nc.tensor.matmul(
                        ps[:, 0:QCH],
                        lhsT=wq1_sb[:, hp * 128 : hp * 128 + 128],
                        rhs=xq1_sb[:, qs],
                        start=False,
                        stop=True,
                    )
                nc.vector.tensor_copy(out=qt_sb[:, hp, qs], in_=ps[:, 0:QCH])

            def kproj(hp, ko):
                """kt[:, hp, off:off+n] for key chunk ko."""
                off, n = KOFF[ko]
                ps = cpool.tile([128, 1024], f32, tag="c")
                for kc in range(KC8):
                    nc.tensor.matmul(
                        ps[:, 0:n],
                        lhsT=wk_sb[:, hp, kc, :],
                        rhs=xkv_sb[:, kc, off : off + n],
                        start=(kc == 0),
                        stop=(not with_bias and kc == KC8 - 1),
                    )
                if with_bias:
                    nc.tensor.matmul(
                        ps[:, 0:n],
                        lhsT=wk1_sb[:, hp * 128 : hp * 128 + 128],
                        rhs=xkv1_sb[:, off : off + n],
                        start=False,
                        stop=True,
                    )
                nc.vector.tensor_copy(
                    out=kt_sb[:, hp, off : off + n], in_=ps[:, 0:n]
                )

            def vproj(mt):
                """v[keys mt*128:+128, all 8 heads] into v_sb."""
                ps = cpool.tile([128, 1024], f32, tag="c")
                for kc in range(KC8):
                    nc.tensor.matmul(
                        ps[:, 0:DL],
                        lhsT=xkv_sb[:, kc, mt * 128 : mt * 128 + 128],
                        rhs=wv_sb[:, kc, :],
                        start=(kc == 0),
                        stop=(not with_bias and kc == KC8 - 1),
                    )
                if with_bias:
                    nc.tensor.matmul(
                        ps[:, 0:DL],
                        lhsT=xkv1_sb[:, mt * 128 : mt * 128 + 128],
                        rhs=wv1_sb[:],
                        start=False,
                        stop=True,
                    )
                nc.vector.tensor_copy(
                    out=v_sb[:, mt, :, 0:64],
                    in_=ps[:, 0:DL].rearrange("p (h e) -> p h e", h=HL),
                )

            def outproj(rt):
                """out rows rt*128:+128 = sum_khp ctx[khp].T @ wo[khp]."""
                rs = slice(rt * 128, rt * 128 + 128)
                ps = cpool.tile([128, 1024], f32, tag="c")
                for nj in range(D // 512):
                    ns = slice(nj * 512, nj * 512 + 512)
                    for khp in range(N_HP):
                        nc.tensor.matmul(
                            ps[:, ns],
                            lhsT=ctx_sb[:, khp, rs],
                            rhs=wo_sb[:, khp, ns],
                            start=(khp == 0),
                            stop=(khp == N_HP - 1),
                        )
                ob = obp.tile([128, D], b16, tag="ob")
                nc.vector.tensor_copy(out=ob[:], in_=ps[:])
                nc.sync.dma_start(out_d[rs, :], ob[:])

            # granule bookkeeping: ensure() emits on demand; the filler deque
            # paces the rest into the attention spine.
            done = set()
            FN = {"q": qproj, "k": kproj, "v": vproj, "o": outproj}

            def ensure(kind, *a):
                key = (kind,) + a
                if key not in done:
                    done.add(key)
                    FN[kind](*a)

            fillers = []

            def drain(n):
                while n > 0 and fillers:
                    key = fillers.pop(0)
                    if key in done:
                        continue
                    done.add(key)
                    FN[key[0]](*key[1:])
                    n -= 1

            # lead-in: enough q/k for the first scores
            ensure("k", 0, 0)
            ensure("q", 0, 0)

            # filler order ~ by deadline. v chunks first (ctx consumes them
            # from ~iteration PIPE of hp0), k chunks for hp0 next, then later
            # head-pairs' q/k, then q for later q-chunks.
            fillers.extend(
                [("v", 0), ("v", 1), ("k", 0, 1), ("v", 2), ("v", 3)]
                + ([("k", 0, 2)] if len(KOFF) > 2 else [])
                + [("v", 4), ("v", 5), ("v", 6), ("q", 1, 0), ("v", 7)]
                + [("v", mt) for mt in range(8, nkc)]
            )
            for hp in (1, 2, 3):
                for ko in range(len(KOFF)):
                    fillers.append(("k", hp, ko))
                if hp < 3:
                    fillers.append(("q", hp + 1, 0))
            for qc in (1, 2, 3):
                for hp in range(N_HP):
                    fillers.append(("q", hp, qc))

            # ---- attention spine: one software pipeline across all blocks.
            # pending holds ctx-pair work and per-block norm markers; the
            # pipeline never flushes at block boundaries, so ACT keeps a
            # steady diet of exp tiles while the PE weaves ctx/norm/filler.
            pending = []  # entries: ("ctx", fn, e01, kc) | ("norm", fn)

            def pump():
                while sum(1 for p in pending if p[0] == "ctx") >= PIPE:
                    ent = pending.pop(0)
                    if ent[0] == "ctx":
                        ent[1](ent[2], ent[3])
                    else:
                        ent[1]()

            for qc in range(NQC):
                qs = slice(qc * QCH, qc * QCH + QCH)
                for hp in range(N_HP):
                    ensure("q", hp, qc)
                    blk = {}

                    def ctx_mm(e01_p, kc_p, blk=blk, hp=hp):
                        ensure("v", kc_p)
                        if "cc" not in blk:
                            blk["cc"] = cpool.tile(
                                [128, 1024], f32, tag="c", name="cc"
                            )
                        cc = blk["cc"]
                        nc.tensor.matmul(
                            cc[:, 0:QCH],
                            lhsT=v_sb[:, kc_p, 2 * hp, :],
                            rhs=e01_p[:, 0:QCH],
                            start=(kc_p == 0),
                            stop=(kc_p == nkc - 1),
                        )
                        nc.tensor.matmul(
                            cc[:, QCH : 2 * QCH],
                            lhsT=v_sb[:, kc_p, 2 * hp + 1, :],
                            rhs=e01_p[:, QCH : 2 * QCH],
                            start=(kc_p == 0),
                            stop=(kc_p == nkc - 1),
                        )

                    def norm(blk=blk, hp=hp, qs=qs, qc=qc):
                        # rows 64-127 of each half hold the denominator
                        # (replicated). reciprocal_approx_fast needs base
                        # partition 0, so relocate it with a plain copy first.
                        cc = blk["cc"]
                        for h in (0, 1):
                            ch = cc[:, h * QCH : (h + 1) * QCH]
                            den = wp.tile([64, QCH], f32, tag="den")
                            nc.vector.tensor_copy(out=den[:], in_=ch[64:128, :])
                            rc = wp.tile([64, QCH], f32, tag="rc")
                            nc.vector.reciprocal_approx_fast(rc[:], den[:])
                            nc.vector.tensor_mul(
                                out=ctx_sb[h * 64 : h * 64 + 64, hp, qs],
                                in0=ch[0:64, :],
                                in1=rc[:],
                            )
                        # out-projection of the previous q-chunk's row tile hp
                        if qc > 0:
                            ensure("o", (qc - 1) * (QCH // 128) + hp)

                    for kc in range(nkc):
                        # emit ctx pops + filler BEFORE the scores pair so the
                        # PE has ordered work to chew while the pair waits on
                        # its s-slot (freed by exp kc-2); otherwise the
                        # scheduler wedges that work BETWEEN the pair's two
                        # row-group matmuls, which stalls ACT on the late half.
                        pump()
                        if qc == 0:
                            drain(2)
                        elif (kc & 1) == 0:
                            drain(1)
                        ensure("k", hp, (kc * 128) // 512)
                        ks = slice(kc * 128, kc * 128 + 128)
                        s01 = spool.tile([128, 2 * QCH], f32, tag="s")
                        with tc.high_priority():
                            nc.tensor.matmul(
                                s01[:, 0:QCH],
                                lhsT=kt_sb[0:64, hp, ks],
                                rhs=qt_sb[0:64, hp, qs],
                            )
                            nc.tensor.matmul(
                                s01[:, QCH : 2 * QCH],
                                lhsT=kt_sb[64:128, hp, ks],
                                rhs=qt_sb[64:128, hp, qs],
                            )
                        e01 = ep.tile([128, 2 * QCH], b16, tag="e")
                        nc.scalar.activation(
                            e01[:],
                            s01[:],
                            EXP,
                            bias=mb_sb[:, kc : kc + 1],
                            scale=SCALE,
                        )
                        pending.append(("ctx", ctx_mm, e01, kc))
                    pending.append(("norm", norm))

            # drain the pipeline, stragglers, and the last q-chunk's rows
            for ent in pending:
                if ent[0] == "ctx":
                    ent[1](ent[2], ent[3])
                else:
                    ent[1]()
            drain(len(fillers))
            for rt in range((NQC - 1) * (QCH // 128), NQC * (QCH // 128)):
                ensure("o", rt)

    nc.finalize()
    return nc


def _host_prep(x, mask, wq, bq, wk, bk, wv, bv, wo):
    x = np.asarray(x, dtype=np.float32)
    mask = np.asarray(mask)
    # per-batch gather of unmasked keys
    idxs = [np.nonzero(mask[b])[0] for b in range(B)]
    nmax = max(1, max(len(i) for i in idxs))
    nkv = min(S, ((nmax + 127) // 128) * 128)
    nkc = nkv // 128
    with_bias = bool(
        np.any(np.asarray(bq)) or np.any(np.asarray(bk)) or np.any(np.asarray(bv))
    )

    in_maps = []
    for c in range(DP * TP):
        b, g = c // TP, c % TP
        sl = slice(g * DL, g * DL + DL)

        idx = idxs[b]
        xg = np.zeros((nkv, D), dtype=np.float32)
        xg[: len(idx)] = x[b][idx]

        mbias = np.full((nkv,), NEG, dtype=np.float32)
        mbias[: len(idx)] = 0.0

        # xq: [128, qc, kc, 512] partition-major
        xqp = np.ascontiguousarray(
            x[b].reshape(NQC, QCH, KC8, 128).transpose(3, 0, 2, 1)
        ).reshape(128, -1).astype(bf16)
        # xkv: [128, kc, nkv]
        xkvp = np.ascontiguousarray(
            xg.reshape(nkv, KC8, 128).transpose(2, 1, 0)
        ).reshape(128, -1).astype(bf16)
        # wq/wk: [128, hp, kc, 128] (w.T chunked partition-major, hp-major)
        def packw(w):
            t = np.asarray(w)[sl, :].T  # [D, DL]
            return np.ascontiguousarray(
                t.reshape(KC8, 128, N_HP, 128).transpose(1, 2, 0, 3)
            ).reshape(128, -1).astype(bf16)
        # wv: [128, kc, DL]
        wvp = np.ascontiguousarray(
            np.asarray(wv)[sl, :].T.reshape(KC8, 128, DL).transpose(1, 0, 2)
        ).reshape(128, -1).astype(bf16)
        # wo: [128, DL//128, D]
        wop = np.ascontiguousarray(
            np.asarray(wo)[:, sl].T.reshape(DL // 128, 128, D).transpose(1, 0, 2)
        ).reshape(128, -1).astype(bf16)

        im = {
            "xq": xqp,
            "xkv": xkvp,
            "wqt": packw(wq),
            "wkt": packw(wk),
            "wvt": wvp,
            "wot": wop,
            "mbias": np.ascontiguousarray(mbias.reshape(nkc, 128).T),
        }
        if with_bias:
            im["bq"] = np.asarray(bq)[None, sl].astype(bf16)
            im["bk"] = np.asarray(bk)[None, sl].astype(bf16)
            im["bv"] = np.asarray(bv)[None, sl].astype(bf16)
        in_maps.append(im)
    return nkv, with_bias, in_maps


def kernel(x, mask, wq, bq, wk, bk, wv, bv, wo, bo):
    from concourse.bass_utils import run_bass_kernel_spmd

    nkv, with_bias, in_maps = _host_prep(x, mask, wq, bq, wk, bk, wv, bv, wo)
    nc = _build(nkv, with_bias)
    res = run_bass_kernel_spmd(nc, in_maps, core_ids=list(range(DP * TP)))

    out = np.empty((B, S, D), dtype=np.float32)
    bo = np.asarray(bo, dtype=np.float32)
    for b in range(B):
        out[b] = (
            res.results[b * TP]["out"].astype(np.float32)
            + res.results[b * TP + 1]["out"].astype(np.float32)
            + bo
        )
    return out
